# revision 2
# baseline (speedup 1.0000x reference)
"""Trainium2 Bass kernel for multi-head attention (Llama-style, GQA 32q/8kv,
RoPE, non-causal softmax as in the source module) distributed over 8
NeuronCores.

Distribution (token-parallel: cores 0-3 own batch 0, cores 4-7 batch 1;
each core owns a 512-token slice and produces its output rows directly —
NO collective sits on the critical path's tail):

  phase A: each core projects k/v for ITS 512 tokens, all 8 kv-heads
           (RoPE'd kT [hd, tok] / transposed v [tok, hd]), writes both into
           one DRAM buffer, then a single AllGather within the 4-core batch
           group replicates the full 2048-token k/v to every core.  The
           gather (~2MB egress) hides behind the start of phase BC.
  phase BC (merged): per q-head h: q-proj for own tokens (wq col chunk
           streamed) -> RoPE -> scores vs all 2048 keys -> exp on ScalarE
           (softmax scale folded) -> P@V accumulated in PSUM -> normalize
           with denominators built on DVE + GPSIMD partition-reduce.
           Merging projection and attention keeps ScalarE's exp (~300us of
           ACT work) under the ~440us of PE work in this phase, instead of
           exp bounding a separate attention phase.
  phase D: o_proj of the own-token attention output against the FULL wo
           (streamed in 4MB column chunks); output is a disjoint
           [512, 4096] fp32 slice, host concatenates.  No exchange needed:
           every head of the own tokens is already local.

All matmuls run in bf16 with fp32 PSUM accumulation (fp32 matmul is 1/4 rate
on TRN2). The RoPE even/odd pairing is turned into contiguous 64-partition
blocks by permuting wq/wk columns on the host (scores are invariant to any
head-dim permutation applied to both q and k).
"""

import math
from contextlib import ExitStack
from dataclasses import dataclass

import numpy as np
import ml_dtypes

import concourse.bass as bass
import concourse.bass_isa as bass_isa
import concourse.mybir as mybir
import concourse.tile as tile
from concourse import bacc
from concourse.masks import make_identity

BF16 = mybir.dt.bfloat16
F32 = mybir.dt.float32
AF = mybir.ActivationFunctionType


@dataclass(frozen=True)
class Cfg:
    B: int = 2
    T: int = 2048          # sequence length (per batch)
    D: int = 4096          # model dim
    H: int = 32            # query heads
    HKV: int = 8           # kv heads
    HD: int = 128          # head dim (must be 128)
    NC: int = 8            # cores
    PREQ: int = 6          # q-heads projected ahead of the attention loop

    @property
    def GRP(self):
        return self.NC // self.B   # cores per batch group

    @property
    def TS(self):
        return self.T // self.GRP  # own token slice per core

    @property
    def KD(self):
        return self.D // 128       # contraction tiles over D

    @property
    def NST(self):
        return self.T // 128       # s-tiles per batch


FULL = Cfg()


def build_nc(cfg: Cfg = FULL, collective: bool = True) -> bass.Bass:
    """Build the SPMD per-core Bass program (identical on all cores).

    collective=False replaces the AllGather with an identity that reads the
    send buffer replicated (wrong results; single-core profiling only).
    """
    B, T, D, HD, NC = cfg.B, cfg.T, cfg.D, cfg.HD, cfg.NC
    H, HKV, KD, TS, GRP = cfg.H, cfg.HKV, cfg.KD, cfg.TS, cfg.GRP
    NREP = H // HKV
    assert HD == 128 and D % 128 == 0 and TS % 128 == 0

    nc = bacc.Bacc(
        "TRN2",
        target_bir_lowering=False,
        debug=False,
        num_devices=NC,
    )

    # ---- kernel I/O (per core) ----
    xT = nc.declare_dram_parameter("xT", [D, TS], BF16, isOutput=False)
    wq = nc.declare_dram_parameter("wq", [D, H * HD], BF16, isOutput=False)
    wk = nc.declare_dram_parameter("wk", [D, HKV * HD], BF16, isOutput=False)
    wv = nc.declare_dram_parameter("wv", [D, HKV * HD], BF16, isOutput=False)
    wo = nc.declare_dram_parameter("wo", [H * HD, D], BF16, isOutput=False)
    cosT = nc.declare_dram_parameter("cosT", [64, TS], F32, isOutput=False)
    sinT = nc.declare_dram_parameter("sinT", [64, TS], F32, isOutput=False)
    out = nc.declare_dram_parameter("out", [TS, D], F32, isOutput=True)

    # tiled DRAM views: [p, ko, free]
    xT_v = xT.rearrange("(ko p) t -> p ko t", p=128)
    wq_v = wq.rearrange("(ko p) m -> p ko m", p=128)
    wk_v = wk.rearrange("(ko p) m -> p ko m", p=128)
    wv_v = wv.rearrange("(ko p) m -> p ko m", p=128)
    wo_v = wo.rearrange("(ko p) d -> p ko d", p=128)

    scale = 1.0 / math.sqrt(HD)

    with ExitStack() as ctx:
        tc = ctx.enter_context(tile.TileContext(nc))

        per = ctx.enter_context(tc.tile_pool(name="per", bufs=1))
        dram = ctx.enter_context(tc.tile_pool(name="dram", bufs=1, space="DRAM"))
        ident_sb = per.tile([128, 128], BF16)
        make_identity(nc, ident_sb[:])
        cos_sb = per.tile([64, TS], F32)
        sin_sb = per.tile([64, TS], F32)
        nc.sync.dma_start(cos_sb[:], cosT[:])
        nc.sync.dma_start(sin_sb[:], sinT[:])

        # AllGather payload: [0] = kT rows (kvh*128+hd), [1] = v rows
        # (flattened [tok, kv*hd] as [tok*2 + half, 512]).
        agsend = dram.tile([2, HKV * HD, TS], BF16)
        agout = dram.tile([GRP, 2, HKV * HD, TS], BF16)
        # token-natural view of the v half: [b, tok, kv*hd]
        send_v = agsend.rearrange("b (tk two) c -> b tk (two c)", two=2)
        agout_v = agout.rearrange("s b (tk two) c -> s b tk (two c)", two=2)

        def rope_apply(dst, psum, rope_pool):
            """psum [128, TS] fp32 (evens on parts 0:64, odds 64:128)
            -> dst [128, TS] bf16, RoPE'd with the own-token cos/sin."""
            qe = psum[0:64, :]
            qo = psum[64:128, :]
            t0 = rope_pool.tile([64, TS], F32, tag="ropetmp0")
            t1 = rope_pool.tile([64, TS], F32, tag="ropetmp1")
            nc.vector.tensor_mul(t0[:], qe, cos_sb[:])
            nc.vector.tensor_mul(t1[:], qo, sin_sb[:])
            nc.vector.tensor_sub(dst[0:64, :], t0[:], t1[:])
            t2 = rope_pool.tile([64, TS], F32, tag="ropetmp0")
            t3 = rope_pool.tile([64, TS], F32, tag="ropetmp1")
            nc.vector.tensor_mul(t2[:], qe, sin_sb[:])
            nc.vector.tensor_mul(t3[:], qo, cos_sb[:])
            nc.vector.tensor_add(dst[64:128, :], t2[:], t3[:])

        # persistent through BC+D: attention output per head, oT [hd, tok]
        oT_pool = ctx.enter_context(tc.tile_pool(name="oT", bufs=1))
        oT_sb = [oT_pool.tile([128, TS], BF16, name=f"oT{h}") for h in range(H)]

        with ExitStack() as ctx_bc:
            xp = ctx_bc.enter_context(tc.tile_pool(name="xp", bufs=1))
            x_sb = xp.tile([128, KD, TS], BF16)
            for kg in range(0, KD, 8):
                nc.sync.dma_start(x_sb[:, kg:kg + 8, :], xT_v[:, kg:kg + 8, :])

            qw = ctx_bc.enter_context(tc.tile_pool(name="qw", bufs=3))
            qh_pool = ctx_bc.enter_context(tc.tile_pool(name="qh", bufs=cfg.PREQ + 3))
            kvp = ctx_bc.enter_context(tc.tile_pool(name="kvp", bufs=2))
            ep = ctx_bc.enter_context(tc.tile_pool(name="ep", bufs=4))
            dnm = ctx_bc.enter_context(tc.tile_pool(name="dnm", bufs=2))
            rope_bc = ctx_bc.enter_context(tc.tile_pool(name="ropebc", bufs=3))

            # ---------- phase A: kv projection on own tokens ----------
            with tc.tile_pool(name="aw", bufs=2) as aw, \
                 tc.tile_pool(name="aps", bufs=2, space="PSUM") as aps, \
                 tc.tile_pool(name="atr", bufs=2, space="PSUM") as atr, \
                 tc.tile_pool(name="asb", bufs=3) as asb:
                for kvh in range(HKV):
                    wkc = aw.tile([128, KD, HD], BF16, tag="wkc")
                    nc.sync.dma_start(wkc[:], wk_v[:, :, kvh * HD:(kvh + 1) * HD])
                    wvc = aw.tile([128, KD, HD], BF16, tag="wvc")
                    nc.sync.dma_start(wvc[:], wv_v[:, :, kvh * HD:(kvh + 1) * HD])

                    pk = aps.tile([128, TS], F32, tag="pkv")
                    for k in range(KD):
                        nc.tensor.matmul(
                            pk[:], lhsT=wkc[:, k, :], rhs=x_sb[:, k, :],
                            start=(k == 0), stop=(k == KD - 1),
                        )
                    kh = asb.tile([128, TS], BF16, tag="kh")
                    rope_apply(kh[:], pk, rope_bc)
                    nc.sync.dma_start(
                        agsend[0, kvh * HD:(kvh + 1) * HD, :], kh[:])

                    pv = aps.tile([128, TS], F32, tag="pkv")
                    for k in range(KD):
                        nc.tensor.matmul(
                            pv[:], lhsT=wvc[:, k, :], rhs=x_sb[:, k, :],
                            start=(k == 0), stop=(k == KD - 1),
                        )
                    vT = asb.tile([128, TS], BF16, tag="vT")
                    nc.vector.tensor_copy(vT[:], pv[:])
                    for t in range(TS // 128):
                        pt = atr.tile([128, 128], BF16, tag="pt")
                        nc.tensor.transpose(
                            pt[:], vT[:, t * 128:(t + 1) * 128], ident_sb[:])
                        vb = asb.tile([128, 128], BF16, tag="vb")
                        nc.vector.tensor_copy(vb[:], pt[:])
                        nc.sync.dma_start(
                            send_v[1, t * 128:(t + 1) * 128,
                                   kvh * HD:(kvh + 1) * HD],
                            vb[:])

            if collective:
                groups = [list(range(g * GRP, (g + 1) * GRP))
                          for g in range(NC // GRP)]
                nc.gpsimd.collective_compute(
                    "AllGather",
                    mybir.AluOpType.bypass,
                    replica_groups=groups,
                    ins=[agsend.opt()],
                    outs=[agout.opt()],
                )
            else:
                # profiling only: read own shard replicated
                for s in range(GRP):
                    pass  # agout left unwritten; reads below map to agout

            # ---------- phase BC: q-proj + attention, pipelined by head ----
            with tc.tile_pool(name="pqps", bufs=2, space="PSUM") as pqps, \
                 tc.tile_pool(name="psps", bufs=2, space="PSUM") as psps, \
                 tc.tile_pool(name="pops", bufs=2, space="PSUM") as pops:

                def qproj(h):
                    wqc = qw.tile([128, KD, HD], BF16, tag="wqc")
                    nc.sync.dma_start(wqc[:], wq_v[:, :, h * HD:(h + 1) * HD])
                    pq = pqps.tile([128, TS], F32, tag="pq")
                    for k in range(KD):
                        nc.tensor.matmul(
                            pq[:], lhsT=wqc[:, k, :], rhs=x_sb[:, k, :],
                            start=(k == 0), stop=(k == KD - 1),
                        )
                    qh = qh_pool.tile([128, TS], BF16, tag="qh")
                    rope_apply(qh[:], pq, rope_bc)
                    return qh

                def load_kv(kvh):
                    src = agout if collective else agsend
                    kT = kvp.tile([128, T], BF16, tag="kT")
                    vsb = kvp.tile([128, cfg.NST, HD], BF16, tag="vsb")
                    for s in range(GRP):
                        if collective:
                            ksrc = agout[s, 0, kvh * HD:(kvh + 1) * HD, :]
                        else:
                            ksrc = agsend[0, kvh * HD:(kvh + 1) * HD, :]
                        nc.sync.dma_start(kT[:, s * TS:(s + 1) * TS], ksrc)
                        for t in range(TS // 128):
                            if collective:
                                vsrc = agout_v[s, 1, t * 128:(t + 1) * 128,
                                               kvh * HD:(kvh + 1) * HD]
                            else:
                                vsrc = send_v[1, t * 128:(t + 1) * 128,
                                              kvh * HD:(kvh + 1) * HD]
                            nc.sync.dma_start(
                                vsb[:, s * (TS // 128) + t, :], vsrc)
                    return kT, vsb

                qhs = {}
                for h in range(cfg.PREQ):
                    qhs[h] = qproj(h)
                kv_cur = load_kv(0)
                kv_nxt = None

                for h in range(H):
                    kvh = h // NREP
                    if h % NREP == 0 and h > 0:
                        kv_cur = kv_nxt
                    kT, vsb = kv_cur
                    qh = qhs.pop(h)

                    po = pops.tile([128, TS], F32, tag="po")
                    tmps = []
                    NSG = cfg.NST // 2
                    for sg in range(NSG):
                        ps = psps.tile([128, 2 * TS], F32, tag="ps")
                        for j in range(2):
                            st = sg * 2 + j
                            nc.tensor.matmul(
                                ps[:, j * TS:(j + 1) * TS],
                                lhsT=kT[:, st * 128:(st + 1) * 128],
                                rhs=qh[:],
                                start=True, stop=True,
                            )
                        e = ep.tile([128, 2 * TS], BF16, tag="e")
                        nc.scalar.activation(e[:], ps[:], AF.Exp, scale=scale)
                        for j in range(2):
                            st = sg * 2 + j
                            nc.tensor.matmul(
                                po[:],
                                lhsT=vsb[:, st, :],
                                rhs=e[:, j * TS:(j + 1) * TS],
                                start=(st == 0), stop=(st == cfg.NST - 1),
                            )
                        tmp = dnm.tile([128, TS], BF16, tag="dtmp",
                                       bufs=NSG + 1, name=f"dt{sg}")
                        nc.vector.tensor_add(
                            tmp[:], e[:, 0:TS], e[:, TS:2 * TS])
                        tmps.append(tmp)
                        # prefetch: next kv block + next q head, mid-head
                        if sg == 2:
                            if h % NREP == 2 and kvh + 1 < HKV:
                                kv_nxt = load_kv(kvh + 1)
                            if h + cfg.PREQ < H:
                                qhs[h + cfg.PREQ] = qproj(h + cfg.PREQ)
                    # fp32 tree over the NSG partials
                    while len(tmps) > 1:
                        nxt = []
                        for i in range(0, len(tmps) - 1, 2):
                            s_ = dnm.tile([128, TS], F32, tag="dtree",
                                          bufs=6, name="dtr")
                            nc.vector.tensor_add(s_[:], tmps[i][:],
                                                 tmps[i + 1][:])
                            nxt.append(s_)
                        if len(tmps) % 2:
                            nxt.append(tmps[-1])
                        tmps = nxt
                    dall = dnm.tile([128, TS], F32, tag="dall")
                    nc.gpsimd.partition_all_reduce(
                        dall[:], tmps[0][:], channels=128,
                        reduce_op=bass_isa.ReduceOp.add)
                    rcp = dnm.tile([128, TS], F32, tag="rcp")
                    nc.vector.reciprocal_approx_fast(rcp[:], dall[:])
                    nc.vector.tensor_mul(oT_sb[h][:], po[:], rcp[:])

        # ---------- phase D: o_proj on own tokens, full wo ----------
        DC = 512
        NDCH = D // DC
        with tc.tile_pool(name="wop", bufs=2) as wop, \
             tc.tile_pool(name="osb", bufs=3) as osbp, \
             tc.tile_pool(name="pdps", bufs=2, space="PSUM") as pdps:
            KO = (H * HD) // 128
            for dch in range(NDCH):
                woc = wop.tile([128, KO, DC], BF16, tag="woc")
                nc.sync.dma_start(woc[:], wo_v[:, :, dch * DC:(dch + 1) * DC])
                for tt in range(TS // 128):
                    pso = pdps.tile([128, DC], F32, tag="pso")
                    for k in range(KO):
                        nc.tensor.matmul(
                            pso[:],
                            lhsT=oT_sb[k][:, tt * 128:(tt + 1) * 128],
                            rhs=woc[:, k, :],
                            start=(k == 0), stop=(k == KO - 1),
                        )
                    osb = osbp.tile([128, DC], F32, tag="osb")
                    nc.vector.tensor_copy(osb[:], pso[:])
                    nc.sync.dma_start(
                        out[tt * 128:(tt + 1) * 128,
                            dch * DC:(dch + 1) * DC],
                        osb[:],
                    )

    nc.compile()
    return nc


# ------------------------------------------------------------------
# host-side input prep
# ------------------------------------------------------------------

def _rope_perm(n_heads_cols: int, HD: int) -> np.ndarray:
    """Column permutation: per head, evens first then odds."""
    idx = np.arange(n_heads_cols)
    h = idx // HD
    j = idx % HD
    old = np.where(j < HD // 2, 2 * j, 2 * (j - HD // 2) + 1)
    return h * HD + old


def make_in_maps(inputs: dict, cfg: Cfg = FULL):
    B, T, D, HD, NC = cfg.B, cfg.T, cfg.D, cfg.HD, cfg.NC
    GRP, TS = cfg.GRP, cfg.TS
    bf = ml_dtypes.bfloat16

    x = np.asarray(inputs["x"], np.float32).reshape(B * T, D)
    xT = np.ascontiguousarray(x.T).astype(bf)          # [D, B*T]

    wq = np.asarray(inputs["wq"], np.float32)
    wk = np.asarray(inputs["wk"], np.float32)
    wv = np.asarray(inputs["wv"], np.float32)
    wo = np.asarray(inputs["wo"], np.float32)

    permq = _rope_perm(wq.shape[1], HD)
    permk = _rope_perm(wk.shape[1], HD)
    wq_p = np.ascontiguousarray(wq[:, permq]).astype(bf)
    wk_p = np.ascontiguousarray(wk[:, permk]).astype(bf)
    wv_b = np.ascontiguousarray(wv).astype(bf)
    wo_b = np.ascontiguousarray(wo).astype(bf)

    cos = np.asarray(inputs["freqs_cos"], np.float32)   # [T, 64]
    sin = np.asarray(inputs["freqs_sin"], np.float32)
    cosT = np.ascontiguousarray(cos.T)                  # [64, T]
    sinT = np.ascontiguousarray(sin.T)

    in_maps = []
    for c in range(NC):
        b = c // GRP
        t0 = (c % GRP) * TS                       # batch-local token start
        g0 = b * T + t0                           # global token start
        in_maps.append({
            "xT": np.ascontiguousarray(xT[:, g0:g0 + TS]),
            "wq": wq_p,
            "wk": wk_p,
            "wv": wv_b,
            "wo": wo_b,
            "cosT": np.ascontiguousarray(cosT[:, t0:t0 + TS]),
            "sinT": np.ascontiguousarray(sinT[:, t0:t0 + TS]),
        })
    return in_maps


_CACHE: dict = {}


def kernel(**inputs) -> np.ndarray:
    cfg = FULL
    sp = inputs.get("start_pos", 0)
    sp = int(np.asarray(sp).reshape(-1)[0]) if np.asarray(sp).size else 0
    assert sp == 0, f"kernel only supports start_pos=0, got {sp}"

    from concourse.bass_utils import run_bass_kernel_spmd

    if "nc" not in _CACHE:
        _CACHE["nc"] = build_nc(cfg)
    nc = _CACHE["nc"]

    in_maps = make_in_maps(inputs, cfg)
    res = run_bass_kernel_spmd(nc, in_maps, list(range(cfg.NC)))
    outs = [res.results[c]["out"] for c in range(cfg.NC)]
    full = np.concatenate(outs, axis=0)          # [B*T, D]
    return full.reshape(cfg.B, cfg.T, cfg.D).astype(np.float32)


if __name__ == "__main__":
    nc = build_nc()
    n = sum(len(bb.instructions) for bb in nc.m.functions[0].blocks)
    print("built", n, "instructions")


# revision 8
# speedup vs baseline: 1.4064x; 1.4064x over previous
"""Trainium2 Bass kernel for multi-head attention (Llama-style, GQA 32q/8kv,
RoPE, non-causal softmax as in the source module) distributed over 8
NeuronCores.

Distribution (token-parallel: cores 0-3 own batch 0, cores 4-7 batch 1;
each core owns a 512-token slice and produces its output rows directly —
NO collective sits on the critical path's tail):

  phase A: each core projects k/v for ITS 512 tokens, all 8 kv-heads
           (RoPE'd kT [hd, tok] / transposed v [tok, hd]), writes both into
           one DRAM buffer, then a single AllGather within the 4-core batch
           group replicates the full 2048-token k/v to every core.  The
           gather (~2MB egress) hides behind the start of phase BC.
  phase BC (merged): per q-head h: q-proj for own tokens (wq col chunk
           streamed) -> RoPE -> scores vs all 2048 keys -> exp on ScalarE
           (softmax scale folded) -> P@V accumulated in PSUM -> normalize
           with denominators built on DVE + GPSIMD partition-reduce.
           Merging projection and attention keeps ScalarE's exp (~300us of
           ACT work) under the ~440us of PE work in this phase, instead of
           exp bounding a separate attention phase.
  phase D: o_proj of the own-token attention output against the FULL wo
           (streamed in 4MB column chunks); output is a disjoint
           [512, 4096] fp32 slice, host concatenates.  No exchange needed:
           every head of the own tokens is already local.  The first wo
           chunk is prefetched during BC and D's PSUM accumulation shares
           the q-proj psum pool, so the scheduler can slide early o_proj
           matmuls into the ACT-bound tail of BC.

All matmuls run in bf16 with fp32 PSUM accumulation (fp32 matmul is 1/4 rate
on TRN2). The RoPE even/odd pairing is turned into contiguous 64-partition
blocks by permuting wq/wk columns on the host (scores are invariant to any
head-dim permutation applied to both q and k).
"""

import math
from contextlib import ExitStack
from dataclasses import dataclass

import numpy as np
import ml_dtypes

import concourse.bass as bass
import concourse.bass_isa as bass_isa
import concourse.mybir as mybir
import concourse.tile as tile
from concourse import bacc
from concourse.masks import make_identity

BF16 = mybir.dt.bfloat16
F32 = mybir.dt.float32
AF = mybir.ActivationFunctionType


@dataclass(frozen=True)
class Cfg:
    B: int = 2
    T: int = 2048          # sequence length (per batch)
    D: int = 4096          # model dim
    H: int = 32            # query heads
    HKV: int = 8           # kv heads
    HD: int = 128          # head dim (must be 128)
    NC: int = 8            # cores
    PREQ: int = 32         # q-heads projected ahead of the attention loop

    @property
    def GRP(self):
        return self.NC // self.B   # cores per batch group

    @property
    def TS(self):
        return self.T // self.GRP  # own token slice per core

    @property
    def KD(self):
        return self.D // 128       # contraction tiles over D

    @property
    def NST(self):
        return self.T // 128       # s-tiles per batch


FULL = Cfg()


def build_nc(cfg: Cfg = FULL, collective: bool = True) -> bass.Bass:
    """Build the SPMD per-core Bass program (identical on all cores).

    collective=False replaces the AllGather with an identity that reads the
    send buffer replicated (wrong results; single-core profiling only).
    """
    B, T, D, HD, NC = cfg.B, cfg.T, cfg.D, cfg.HD, cfg.NC
    H, HKV, KD, TS, GRP = cfg.H, cfg.HKV, cfg.KD, cfg.TS, cfg.GRP
    NREP = H // HKV
    assert HD == 128 and D % 128 == 0 and TS % 128 == 0

    nc = bacc.Bacc(
        "TRN2",
        target_bir_lowering=False,
        debug=False,
        num_devices=NC,
    )

    # ---- kernel I/O (per core) ----
    xT = nc.declare_dram_parameter("xT", [D, TS], BF16, isOutput=False)
    wq = nc.declare_dram_parameter("wq", [D, H * HD], BF16, isOutput=False)
    wk = nc.declare_dram_parameter("wk", [D, HKV * HD], BF16, isOutput=False)
    wv = nc.declare_dram_parameter("wv", [D, HKV * HD], BF16, isOutput=False)
    wo = nc.declare_dram_parameter("wo", [H * HD, D], BF16, isOutput=False)
    cosT = nc.declare_dram_parameter("cosT", [64, TS], F32, isOutput=False)
    sinT = nc.declare_dram_parameter("sinT", [64, TS], F32, isOutput=False)
    out = nc.declare_dram_parameter("out", [TS, D], F32, isOutput=True)

    # tiled DRAM views: [p, ko, free]
    xT_v = xT.rearrange("(ko p) t -> p ko t", p=128)
    wq_v = wq.rearrange("(ko p) m -> p ko m", p=128)
    wk_v = wk.rearrange("(ko p) m -> p ko m", p=128)
    wv_v = wv.rearrange("(ko p) m -> p ko m", p=128)
    wo_v = wo.rearrange("(ko p) d -> p ko d", p=128)

    scale = 1.0 / math.sqrt(HD)

    with ExitStack() as ctx:
        tc = ctx.enter_context(tile.TileContext(nc))

        per = ctx.enter_context(tc.tile_pool(name="per", bufs=1))
        dram = ctx.enter_context(tc.tile_pool(name="dram", bufs=1, space="DRAM"))
        ident_sb = per.tile([128, 128], BF16)
        make_identity(nc, ident_sb[:])
        cos_sb = per.tile([64, TS], F32)
        sin_sb = per.tile([64, TS], F32)
        nc.sync.dma_start(cos_sb[:], cosT[:])
        nc.sync.dma_start(sin_sb[:], sinT[:])

        # AllGather payload: [0] = kT rows (kvh*128+hd), [1] = v rows
        # (flattened [tok, kv*hd] as [tok*2 + half, 512]).
        agsend = dram.tile([2, HKV * HD, TS], BF16)
        agout = dram.tile([GRP, 2, HKV * HD, TS], BF16)
        # token-natural view of the v half: [b, tok, kv*hd]
        send_v = agsend.rearrange("b (tk two) c -> b tk (two c)", two=2)
        agout_v = agout.rearrange("s b (tk two) c -> s b tk (two c)", two=2)

        def rope_apply(dst, psum, rope_pool):
            """psum [128, TS] fp32 (evens on parts 0:64, odds 64:128)
            -> dst [128, TS] bf16, RoPE'd with the own-token cos/sin."""
            qe = psum[0:64, :]
            qo = psum[64:128, :]
            t0 = rope_pool.tile([64, TS], F32, tag="ropetmp0")
            t1 = rope_pool.tile([64, TS], F32, tag="ropetmp1")
            nc.vector.tensor_mul(t0[:], qe, cos_sb[:])
            nc.vector.tensor_mul(t1[:], qo, sin_sb[:])
            nc.vector.tensor_sub(dst[0:64, :], t0[:], t1[:])
            t2 = rope_pool.tile([64, TS], F32, tag="ropetmp0")
            t3 = rope_pool.tile([64, TS], F32, tag="ropetmp1")
            nc.vector.tensor_mul(t2[:], qe, sin_sb[:])
            nc.vector.tensor_mul(t3[:], qo, cos_sb[:])
            nc.vector.tensor_add(dst[64:128, :], t2[:], t3[:])

        # persistent through BC+D: attention output per head, oT [hd, tok]
        oT_pool = ctx.enter_context(tc.tile_pool(name="oT", bufs=1))
        oT_sb = [oT_pool.tile([128, TS], BF16, name=f"oT{h}") for h in range(H)]
        osbp = ctx.enter_context(tc.tile_pool(name="osb", bufs=3))
        KO = (H * HD) // 128
        DC = 512
        DC1 = 256          # early o_proj chunks (hoistable into BC's tail)
        qw_bufs = 1 if cfg.PREQ >= 24 else 2
        rope_bufs = 2 if cfg.PREQ >= 24 else 3
        # shared psum pool: q-proj accumulators during BC, o_proj during D
        pq_ps = ctx.enter_context(tc.tile_pool(name="pqps", bufs=2, space="PSUM"))

        with ExitStack() as ctx_bc:
            xp = ctx_bc.enter_context(tc.tile_pool(name="xp", bufs=1))
            x_sb = xp.tile([128, KD, TS], BF16)
            for kg in range(0, KD, 8):
                nc.sync.dma_start(x_sb[:, kg:kg + 8, :], xT_v[:, kg:kg + 8, :])

            qw = ctx_bc.enter_context(tc.tile_pool(name="qw", bufs=qw_bufs))
            qh_pool = ctx_bc.enter_context(
                tc.tile_pool(name="qh", bufs=cfg.PREQ + 2))
            kvp = ctx_bc.enter_context(tc.tile_pool(name="kvp", bufs=2))
            ep = ctx_bc.enter_context(tc.tile_pool(name="ep", bufs=4))
            dnm = ctx_bc.enter_context(tc.tile_pool(name="dnm", bufs=3))
            rope_bc = ctx_bc.enter_context(tc.tile_pool(name="ropebc", bufs=rope_bufs))

            # ---------- phase A: kv projection on own tokens ----------
            # k-loop first, then v-loop: halves the weight-DMA pressure at
            # kernel start (only wk streams during the first 55us).
            with tc.tile_pool(name="aw", bufs=2) as aw, \
                 tc.tile_pool(name="aps", bufs=2, space="PSUM") as aps, \
                 tc.tile_pool(name="atr", bufs=2, space="PSUM") as atr, \
                 tc.tile_pool(name="asb", bufs=2) as asb:
                for kvh in range(HKV):
                    wkc = aw.tile([128, KD, HD], BF16, tag="wkc")
                    for kg in range(0, KD, 8):
                        nc.sync.dma_start(
                            wkc[:, kg:kg + 8, :],
                            wk_v[:, kg:kg + 8, kvh * HD:(kvh + 1) * HD])
                    pk = aps.tile([128, TS], F32, tag="pkv")
                    for k in range(KD):
                        nc.tensor.matmul(
                            pk[:], lhsT=wkc[:, k, :], rhs=x_sb[:, k, :],
                            start=(k == 0), stop=(k == KD - 1),
                        )
                    kh = asb.tile([128, TS], BF16, tag="kh")
                    rope_apply(kh[:], pk, rope_bc)
                    nc.sync.dma_start(
                        agsend[0, kvh * HD:(kvh + 1) * HD, :], kh[:])

                pend_tr = None
                for kvh in range(HKV):
                    wvc = aw.tile([128, KD, HD], BF16, tag="wvc")
                    for kg in range(0, KD, 8):
                        nc.sync.dma_start(
                            wvc[:, kg:kg + 8, :],
                            wv_v[:, kg:kg + 8, kvh * HD:(kvh + 1) * HD])
                    pv = aps.tile([128, TS], F32, tag="pkv")
                    for k in range(KD):
                        nc.tensor.matmul(
                            pv[:], lhsT=wvc[:, k, :], rhs=x_sb[:, k, :],
                            start=(k == 0), stop=(k == KD - 1),
                        )
                    vT = asb.tile([128, TS], BF16, tag="vT")
                    nc.vector.tensor_copy(vT[:], pv[:])
                    # defer transposes one kvh so PE isn't waiting on the
                    # psum->sbuf copy of the vT it just produced
                    if pend_tr is not None:
                        _vT, _kvh = pend_tr
                        for t in range(TS // 128):
                            pt = atr.tile([128, 128], BF16, tag="pt")
                            nc.tensor.transpose(
                                pt[:], _vT[:, t * 128:(t + 1) * 128],
                                ident_sb[:])
                            vb = asb.tile([128, 128], BF16, tag="vb", bufs=4)
                            nc.vector.tensor_copy(vb[:], pt[:])
                            nc.sync.dma_start(
                                send_v[1, t * 128:(t + 1) * 128,
                                       _kvh * HD:(_kvh + 1) * HD],
                                vb[:])
                    pend_tr = (vT, kvh)
                _vT, _kvh = pend_tr
                for t in range(TS // 128):
                    pt = atr.tile([128, 128], BF16, tag="pt")
                    nc.tensor.transpose(
                        pt[:], _vT[:, t * 128:(t + 1) * 128], ident_sb[:])
                    vb = asb.tile([128, 128], BF16, tag="vb", bufs=4)
                    nc.vector.tensor_copy(vb[:], pt[:])
                    nc.sync.dma_start(
                        send_v[1, t * 128:(t + 1) * 128,
                               _kvh * HD:(_kvh + 1) * HD],
                        vb[:])

            if collective:
                groups = [list(range(g * GRP, (g + 1) * GRP))
                          for g in range(NC // GRP)]
                nc.gpsimd.collective_compute(
                    "AllGather",
                    mybir.AluOpType.bypass,
                    replica_groups=groups,
                    ins=[agsend.opt()],
                    outs=[agout.opt()],
                )


            # ---------- phase BC: q-proj + attention, pipelined by head ----
            with tc.tile_pool(name="psps", bufs=2, space="PSUM") as psps, \
                 tc.tile_pool(name="pops", bufs=2, space="PSUM") as pops:

                def qproj(h):
                    wqc = qw.tile([128, KD, HD], BF16, tag="wqc")
                    nc.sync.dma_start(wqc[:], wq_v[:, :, h * HD:(h + 1) * HD])
                    pq = pq_ps.tile([128, TS], F32, tag="pq")
                    for k in range(KD):
                        nc.tensor.matmul(
                            pq[:], lhsT=wqc[:, k, :], rhs=x_sb[:, k, :],
                            start=(k == 0), stop=(k == KD - 1),
                        )
                    qh = qh_pool.tile([128, TS], BF16, tag="qh")
                    rope_apply(qh[:], pq, rope_bc)
                    return qh

                def load_kv(kvh):
                    kT = kvp.tile([128, T], BF16, tag="kT")
                    vsb = kvp.tile([128, cfg.NST, HD], BF16, tag="vsb")
                    for s in range(GRP):
                        if collective:
                            ksrc = agout[s, 0, kvh * HD:(kvh + 1) * HD, :]
                        else:
                            ksrc = agsend[0, kvh * HD:(kvh + 1) * HD, :]
                        nc.sync.dma_start(kT[:, s * TS:(s + 1) * TS], ksrc)
                        for t in range(TS // 128):
                            if collective:
                                vsrc = agout_v[s, 1, t * 128:(t + 1) * 128,
                                               kvh * HD:(kvh + 1) * HD]
                            else:
                                vsrc = send_v[1, t * 128:(t + 1) * 128,
                                              kvh * HD:(kvh + 1) * HD]
                            nc.sync.dma_start(
                                vsb[:, s * (TS // 128) + t, :], vsrc)
                    return kT, vsb

                qhs = {}
                for h in range(cfg.PREQ):
                    qhs[h] = qproj(h)
                kv_cur = load_kv(0)
                kv_nxt = None

                for h in range(H):
                    kvh = h // NREP
                    if h % NREP == 0 and h > 0:
                        kv_cur = kv_nxt
                    kT, vsb = kv_cur
                    qh = qhs.pop(h)

                    po = pops.tile([128, TS], F32, tag="po")
                    acc = None
                    NSG = cfg.NST // 2
                    for sg in range(NSG):
                        ps = psps.tile([128, 2 * TS], F32, tag="ps")
                        for j in range(2):
                            st = sg * 2 + j
                            nc.tensor.matmul(
                                ps[:, j * TS:(j + 1) * TS],
                                lhsT=kT[:, st * 128:(st + 1) * 128],
                                rhs=qh[:],
                                start=True, stop=True,
                            )
                        e = ep.tile([128, 2 * TS], BF16, tag="e")
                        nc.scalar.activation(e[:], ps[:], AF.Exp, scale=scale)
                        for j in range(2):
                            st = sg * 2 + j
                            nc.tensor.matmul(
                                po[:],
                                lhsT=vsb[:, st, :],
                                rhs=e[:, j * TS:(j + 1) * TS],
                                start=(st == 0), stop=(st == cfg.NST - 1),
                            )
                        tmp = dnm.tile([128, TS], BF16, tag="dtmp", bufs=2)
                        nc.vector.tensor_add(
                            tmp[:], e[:, 0:TS], e[:, TS:2 * TS])
                        if acc is None:
                            acc = tmp
                        else:
                            nacc = dnm.tile([128, TS], F32, tag="dacc",
                                            bufs=3)
                            nc.vector.tensor_add(nacc[:], acc[:], tmp[:])
                            acc = nacc
                        # prefetch: next kv block + next q head, mid-head
                        if sg == 2:
                            if h % NREP == 2 and kvh + 1 < HKV:
                                kv_nxt = load_kv(kvh + 1)
                            if h + cfg.PREQ < H:
                                qhs[h + cfg.PREQ] = qproj(h + cfg.PREQ)
                    dall = dnm.tile([128, TS], F32, tag="dall", bufs=2)
                    nc.gpsimd.partition_all_reduce(
                        dall[:], acc[:], channels=128,
                        reduce_op=bass_isa.ReduceOp.add)
                    rcp = dnm.tile([128, TS], F32, tag="rcp", bufs=2)
                    nc.vector.reciprocal_approx_fast(rcp[:], dall[:])
                    nc.vector.tensor_mul(oT_sb[h][:], po[:], rcp[:])

        # ---------- phase D: o_proj on own tokens, full wo ----------
        # two small early chunks from wopA (allocated in the released x/qw
        # zone, so their DMAs depend only on the q projections finishing and
        # the scheduler can slide these matmuls into BC's ACT-bound tail),
        # then 512-col chunks from wopB.
        chunks = [(0, DC1), (DC1, DC1)] + \
            [(2 * DC1 + i * DC, DC) for i in range(7)]
        with tc.tile_pool(name="wopA", bufs=2) as wopA, \
             tc.tile_pool(name="wopB", bufs=2) as wopB:
            for c0, cw in chunks:
                pool = wopA if cw == DC1 else wopB
                woc = pool.tile([128, KO, cw], BF16, tag=f"woc{cw}")
                for kg in range(0, KO, 16):
                    nc.sync.dma_start(
                        woc[:, kg:kg + 16, :],
                        wo_v[:, kg:kg + 16, c0:c0 + cw])
                for tt in range(TS // 128):
                    pso = pq_ps.tile([128, cw], F32, tag="pq")
                    for k in range(KO):
                        nc.tensor.matmul(
                            pso[:],
                            lhsT=oT_sb[k][:, tt * 128:(tt + 1) * 128],
                            rhs=woc[:, k, :],
                            start=(k == 0), stop=(k == KO - 1),
                        )
                    osb = osbp.tile([128, cw], F32, tag=f"osb{cw}")
                    nc.vector.tensor_copy(osb[:], pso[:])
                    nc.sync.dma_start(
                        out[tt * 128:(tt + 1) * 128, c0:c0 + cw],
                        osb[:],
                    )

    nc.compile()
    return nc


# ------------------------------------------------------------------
# host-side input prep
# ------------------------------------------------------------------

def _rope_perm(n_heads_cols: int, HD: int) -> np.ndarray:
    """Column permutation: per head, evens first then odds."""
    idx = np.arange(n_heads_cols)
    h = idx // HD
    j = idx % HD
    old = np.where(j < HD // 2, 2 * j, 2 * (j - HD // 2) + 1)
    return h * HD + old


def make_in_maps(inputs: dict, cfg: Cfg = FULL):
    B, T, D, HD, NC = cfg.B, cfg.T, cfg.D, cfg.HD, cfg.NC
    GRP, TS = cfg.GRP, cfg.TS
    bf = ml_dtypes.bfloat16

    x = np.asarray(inputs["x"], np.float32).reshape(B * T, D)
    xT = np.ascontiguousarray(x.T).astype(bf)          # [D, B*T]

    wq = np.asarray(inputs["wq"], np.float32)
    wk = np.asarray(inputs["wk"], np.float32)
    wv = np.asarray(inputs["wv"], np.float32)
    wo = np.asarray(inputs["wo"], np.float32)

    permq = _rope_perm(wq.shape[1], HD)
    permk = _rope_perm(wk.shape[1], HD)
    wq_p = np.ascontiguousarray(wq[:, permq]).astype(bf)
    wk_p = np.ascontiguousarray(wk[:, permk]).astype(bf)
    wv_b = np.ascontiguousarray(wv).astype(bf)
    wo_b = np.ascontiguousarray(wo).astype(bf)

    cos = np.asarray(inputs["freqs_cos"], np.float32)   # [T, 64]
    sin = np.asarray(inputs["freqs_sin"], np.float32)
    cosT = np.ascontiguousarray(cos.T)                  # [64, T]
    sinT = np.ascontiguousarray(sin.T)

    in_maps = []
    for c in range(NC):
        b = c // GRP
        t0 = (c % GRP) * TS                       # batch-local token start
        g0 = b * T + t0                           # global token start
        in_maps.append({
            "xT": np.ascontiguousarray(xT[:, g0:g0 + TS]),
            "wq": wq_p,
            "wk": wk_p,
            "wv": wv_b,
            "wo": wo_b,
            "cosT": np.ascontiguousarray(cosT[:, t0:t0 + TS]),
            "sinT": np.ascontiguousarray(sinT[:, t0:t0 + TS]),
        })
    return in_maps


_CACHE: dict = {}


def kernel(**inputs) -> np.ndarray:
    cfg = FULL
    sp = inputs.get("start_pos", 0)
    sp = int(np.asarray(sp).reshape(-1)[0]) if np.asarray(sp).size else 0
    assert sp == 0, f"kernel only supports start_pos=0, got {sp}"

    from concourse.bass_utils import run_bass_kernel_spmd

    if "nc" not in _CACHE:
        _CACHE["nc"] = build_nc(cfg)
    nc = _CACHE["nc"]

    in_maps = make_in_maps(inputs, cfg)
    res = run_bass_kernel_spmd(nc, in_maps, list(range(cfg.NC)))
    outs = [res.results[c]["out"] for c in range(cfg.NC)]
    full = np.concatenate(outs, axis=0)          # [B*T, D]
    return full.reshape(cfg.B, cfg.T, cfg.D).astype(np.float32)


if __name__ == "__main__":
    nc = build_nc()
    n = sum(len(bb.instructions) for bb in nc.m.functions[0].blocks)
    print("built", n, "instructions")
